# revision 7
# baseline (speedup 1.0000x reference)
"""Trainium2 Bass kernel for nn_DenseLayer: y = x @ W + b.

x: (1, 8192) f32, W: (8192, 8192) f32, b: (8192,) f32 -> y: (1, 8192) f32.

Sharding: W column-sharded across 8 NeuronCores (1024 output columns each),
x replicated, each core computes its output slice plus local bias slice.

Per-core compute is a memory-bound matvec (32 MB of W per core). To keep
full fp32-level accuracy while streaming W at bf16 matmul rate, W and x are
each split host-side into hi/lo bf16 parts (W = Wh + Wl, x = xh + xl) and
the kernel computes xh@Wh + xl@Wh + xh@Wl with fp32 PSUM accumulation
(the dropped xl@Wl term is ~2^-18 relative). Total HBM traffic per core is
the same 32 MB as fp32 W, so the DMA roofline is unchanged, but the PE
runs at the 1-cycle/row bf16 rate instead of 4 cycles/row for fp32.

Layout trick: the stationary operand packs (xh, xl) as two columns so one
matmul against Wh produces both xh@Wh and xl@Wh in PSUM partitions 0/1;
a second matmul with (xh, 0) against Wl accumulates xh@Wl. The two PSUM
rows are folded (row0 + row1 + bias) at the end.
"""

import numpy as np
import ml_dtypes

IN_LEN = 8192
OUT_LEN = 8192
NCORES = 8
OUT_SLICE = OUT_LEN // NCORES  # 1024 output columns per core
P = 128
KCHUNKS = IN_LEN // P  # 64 contraction chunks of 128
S = 4  # k-chunks per supertile DMA (S*2048 bf16 per partition line)
NST = KCHUNKS // S  # number of supertile DMAs
LINE = S * 2 * OUT_SLICE  # bf16 elements per partition line per supertile
W_BUFS = 4  # supertile double-buffering depth
MM_N = 512  # moving free dim per matmul (one PSUM bank of fp32)
NHALF = OUT_SLICE // MM_N  # output column groups (PSUM banks)

_BF16 = ml_dtypes.bfloat16

_nc_cache = None


def _build():
    import concourse.bass as bass
    import concourse.mybir as mybir
    from concourse.tile import TileContext

    nc = bass.Bass(trn_type="TRN2")

    whl = nc.dram_tensor(
        "whl", [NST, P, LINE], mybir.dt.bfloat16, kind="ExternalInput"
    )
    xs = nc.dram_tensor(
        "xs", [P, KCHUNKS * 4], mybir.dt.bfloat16, kind="ExternalInput"
    )
    bias = nc.dram_tensor(
        "bias", [1, OUT_SLICE], mybir.dt.float32, kind="ExternalInput"
    )
    y = nc.dram_tensor("y", [1, OUT_SLICE], mybir.dt.float32, kind="ExternalOutput")

    with TileContext(nc) as tc:
        with (
            tc.tile_pool(name="wpool", bufs=W_BUFS) as wpool,
            tc.tile_pool(name="spool", bufs=1) as spool,
            tc.tile_pool(name="ppool", bufs=1, space="PSUM") as ppool,
        ):
            xs_t = spool.tile([P, KCHUNKS * 4], mybir.dt.bfloat16, name="xs_t")
            nc.sync.dma_start(xs_t[:, :], xs[:, :])
            b_t = spool.tile([1, OUT_SLICE], mybir.dt.float32, name="b_t")
            nc.sync.dma_start(b_t[:, :], bias[:, :])
            # Stage bias through DVE so the final add's inputs are all
            # DVE-produced (every op may carry at most ONE embedded sync
            # wait in this walrus build; joins of two producers must be
            # chained through single-dependency copies).
            b_s = spool.tile([1, OUT_SLICE], mybir.dt.float32, name="b_s")
            nc.vector.tensor_copy(b_s[:, :], b_t[:, :])

            psums = [
                ppool.tile([2, MM_N], mybir.dt.float32, name=f"ps{h}", tag=f"ps{h}")
                for h in range(NHALF)
            ]

            for st in range(NST):
                wt = wpool.tile([P, LINE], mybir.dt.bfloat16, name="wt", tag="wt")
                # SWDGE (gpsimd) path: HWDGE direct2d DMAs only support one
                # embedded sync-wait, but slot-reuse WAR deps need two+.
                nc.gpsimd.dma_start(wt[:, :], whl[st, :, :])
                for j in range(S):
                    k = st * S + j
                    base = j * 2 * OUT_SLICE
                    for h in range(NHALF):
                        rhs_h = wt[:, base + h * MM_N : base + (h + 1) * MM_N]
                        rhs_l = wt[
                            :,
                            base + OUT_SLICE + h * MM_N : base
                            + OUT_SLICE
                            + (h + 1) * MM_N,
                        ]
                        # (xh, xl) @ Wh -> psum rows 0,1
                        nc.tensor.matmul(
                            psums[h][:, :],
                            xs_t[:, k * 4 : k * 4 + 2],
                            rhs_h,
                            start=(k == 0),
                            stop=False,
                        )
                        # (xh, 0) @ Wl -> psum rows 0,1 (row1 += 0)
                        nc.tensor.matmul(
                            psums[h][:, :],
                            xs_t[:, k * 4 + 2 : k * 4 + 4],
                            rhs_l,
                            start=False,
                            stop=(k == KCHUNKS - 1),
                        )

            out_t = spool.tile([1, OUT_SLICE], mybir.dt.float32, name="out_t")
            for h in range(NHALF):
                # PSUM -> SBUF (partition-aligned; DMA can't read PSUM)
                pcopy = spool.tile(
                    [2, MM_N], mybir.dt.float32, name=f"pc{h}", tag=f"pc{h}"
                )
                nc.vector.tensor_copy(pcopy[:, :], psums[h][:, :])
                # row 1 (partition 1) -> partition 0 via SBUF->SBUF DMA
                fold = spool.tile(
                    [1, MM_N], mybir.dt.float32, name=f"fold{h}", tag=f"fold{h}"
                )
                nc.sync.dma_start(fold[:, :], pcopy[1:2, :])
                # re-stage through DVE so the add below is a DVE-only join
                fold2 = spool.tile(
                    [1, MM_N], mybir.dt.float32, name=f"fold2{h}", tag=f"fold2{h}"
                )
                nc.vector.tensor_copy(fold2[:, :], fold[:, :])
                nc.vector.tensor_add(
                    out_t[:, h * MM_N : (h + 1) * MM_N], pcopy[0:1, :], fold2[:, :]
                )
            nc.vector.tensor_add(out_t[:, :], out_t[:, :], b_s[:, :])
            nc.sync.dma_start(y[:, :], out_t[:, :])

    _strip_redundant_dma_waits(nc)
    _hoist_extra_waits(nc)
    return nc


def _hoist_extra_waits(nc):
    """Split multi-wait instructions for walrus builds that only support one
    embedded sync wait per instruction.

    All but the last wait are hoisted onto wait-only NoOps inserted
    immediately before the instruction in its basic block, on the same
    engine. The engine sequencer processes instructions in order, so every
    hoisted wait is satisfied before the original instruction dispatches.
    """
    import concourse.mybir as mybir

    n = 0
    for blk in nc.m.functions[0].blocks:
        lst = blk.instructions
        i = 0
        while i < len(lst):
            inst = lst[i]
            si = inst.sync_info
            waits = list(si.on_wait) if si and si.on_wait else []
            if len(waits) > 1:
                for w in waits[:-1]:
                    nop = mybir.InstNoOp(
                        name=f"I-waitnop-{n}",
                        engine=inst.engine,
                        sync_info=mybir.SyncInfo(on_wait=[w], on_update=[]),
                    )
                    n += 1
                    nc.register_instruction(nop)
                    lst.insert(i, nop)
                    i += 1
                si.on_wait = [waits[-1]]
            i += 1


def _strip_redundant_dma_waits(nc):
    """Drop transitively-redundant DMASW waits from W-streaming DMAs.

    The walrus codegen DMA template carries at most ONE embedded sync wait,
    but Tile attaches two to each W supertile DMA that reuses an SBUF slot:
    a PE wait (WAR: matmuls that read the old tile) and a DMASW wait (WAW:
    the fill DMA that wrote the old tile). The WAW wait is redundant — the
    matmuls the PE wait covers themselves waited on that fill DMA — but
    Tile's sem pass is not transitively minimal across processors. Verify
    the transitivity explicitly, then strip the DMASW wait.
    """
    fn = nc.m.functions[0]
    # Walk every engine's instruction stream in order, accumulating for each
    # (engine-sem tick) the maximum DMASW/DMAHW sem values observed (waited
    # on) at or before that tick.
    pe_ticks = []  # list of (cum_pe_updates, {lane_name: max_waited_value})
    observed = {}
    cum = 0
    for blk in fn.blocks:
        for inst in blk.instructions:
            si = inst.sync_info
            if si is None:
                continue
            if str(inst.engine) == "EngineType.PE":
                for w in si.on_wait or []:
                    if "DMA" in w.ant_name:
                        observed[w.ant_name] = max(
                            observed.get(w.ant_name, 0), w.wait_value
                        )
                for u in si.on_update or []:
                    if u.ant_name.startswith("PE"):
                        cum += u.update_value
                        pe_ticks.append((cum, dict(observed)))

    def observed_at(pe_value, lane):
        best = 0
        for cumv, obs in pe_ticks:
            if cumv <= pe_value:
                best = max(best, obs.get(lane, 0))
            else:
                break
        return best

    for blk in fn.blocks:
        for inst in blk.instructions:
            if type(inst).__name__ != "InstDMACopy":
                continue
            si = inst.sync_info
            waits = list(si.on_wait or [])
            if len(waits) <= 1:
                continue
            pe_waits = [w for w in waits if w.ant_name.startswith("PE")]
            dma_waits = [w for w in waits if "DMA" in w.ant_name]
            if len(pe_waits) != 1 or len(pe_waits) + len(dma_waits) != len(waits):
                raise RuntimeError(
                    f"unexpected wait mix on {inst.name}: "
                    f"{[(w.ant_name, w.wait_value) for w in waits]}"
                )
            pe_v = pe_waits[0].wait_value
            for w in dma_waits:
                if observed_at(pe_v, w.ant_name) < w.wait_value:
                    raise RuntimeError(
                        f"cannot strip wait {w.ant_name}>={w.wait_value} on "
                        f"{inst.name}: not implied by PE>={pe_v}"
                    )
            si.on_wait = pe_waits


def _get_nc():
    global _nc_cache
    if _nc_cache is None:
        _nc_cache = _build()
    return _nc_cache


def _split_bf16(a):
    """a (f32) -> (hi, lo) bf16 with hi + lo ~= a."""
    hi = a.astype(_BF16)
    lo = (a - hi.astype(np.float32)).astype(_BF16)
    return hi, lo


def _prepare_in_maps(x, W, b):
    x = np.ascontiguousarray(np.asarray(x, dtype=np.float32)).reshape(1, IN_LEN)
    W = np.asarray(W, dtype=np.float32).reshape(IN_LEN, OUT_LEN)
    b = np.ascontiguousarray(np.asarray(b, dtype=np.float32)).reshape(OUT_LEN)

    xh, xl = _split_bf16(x.reshape(IN_LEN))
    xs = np.zeros((P, KCHUNKS, 4), dtype=_BF16)
    xs[:, :, 0] = xh.reshape(KCHUNKS, P).T
    xs[:, :, 1] = xl.reshape(KCHUNKS, P).T
    xs[:, :, 2] = xh.reshape(KCHUNKS, P).T
    xs = np.ascontiguousarray(xs.reshape(P, KCHUNKS * 4))

    in_maps = []
    for c in range(NCORES):
        Wc = np.ascontiguousarray(W[:, c * OUT_SLICE : (c + 1) * OUT_SLICE])
        Wh, Wl = _split_bf16(Wc)
        # [NST, S, P, OUT_SLICE] -> [NST, P, S, OUT_SLICE]
        Wh4 = Wh.reshape(NST, S, P, OUT_SLICE).transpose(0, 2, 1, 3)
        Wl4 = Wl.reshape(NST, S, P, OUT_SLICE).transpose(0, 2, 1, 3)
        # -> [NST, P, S, 2, OUT_SLICE] -> [NST, P, LINE]
        whl = np.stack([Wh4, Wl4], axis=3)
        whl = np.ascontiguousarray(whl).reshape(NST, P, LINE)
        in_maps.append(
            {
                "whl": whl,
                "xs": xs,
                "bias": np.ascontiguousarray(
                    b[c * OUT_SLICE : (c + 1) * OUT_SLICE]
                ).reshape(1, OUT_SLICE),
            }
        )
    return in_maps


def _run(x, W, b, trace=False):
    from concourse.bass_utils import run_bass_kernel_spmd

    nc = _get_nc()
    in_maps = _prepare_in_maps(x, W, b)
    res = run_bass_kernel_spmd(
        nc, in_maps, core_ids=list(range(NCORES)), trace=trace
    )
    y = np.concatenate([r["y"] for r in res.results], axis=1)
    return np.ascontiguousarray(y.astype(np.float32)), res


def kernel(x, W, b):
    y, _ = _run(x, W, b, trace=False)
    return y
